# revision 23
# baseline (speedup 1.0000x reference)
"""DCT-policy sample+scatter kernel for 8 TRN2 NeuronCores.

Reference computation:
    std    = exp(log_std)
    sample = mean + std * eps                       # (N,) with N = C*NH*NW*K
    coeffs = zeros(C*H*W).at[flat_idx].set(sample)  # structured zigzag scatter
    log_prob = -0.5 * sum(eps^2 + 2*log_std + log2pi)
    entropy  = sum(0.5*(1+log2pi) + log_std)

flat_idx is the deterministic zigzag-DCT layout: param (c, bh, bw, k) lands at
output [c, bh*8+u_k, bw*8+v_k] where (u_k, v_k) is the k-th zigzag position of
an 8x8 block (first 16 kept).  Treating the output as (C*NH*8, W) rows, each
block row br=(c,bh) of params (512*16 values) maps into 8 output rows; only
u=0..5 contain nonzeros, and within each 8-wide block the nonzero v's form a
contiguous run starting at v=0.  So the scatter is a fixed affine permutation:
we build the dense output tile in SBUF with a handful of strided vector adds
and DMA contiguous rows out.  Rows u=6,7 are all-zero and never written (the
runner donates zero-initialized output buffers).

Sharding: the flat param dim (and correspondingly the output rows) are split
contiguously across the 8 cores: 192 block rows of params -> 1536 output rows
per core.  Scalar sums are returned as per-partition partials and reduced on
host in float64.
"""

import math
import sys

import numpy as np

for _p in ("/opt/trn_rl_repo",):
    if _p not in sys.path:
        sys.path.insert(0, _p)

# ---------------------------------------------------------------- constants
C, H, W = 3, 4096, 4096
B = 8              # DCT block size
K = 16             # kept zigzag coeffs per block
NH, NW = H // B, W // B        # 512, 512
R = C * NH                     # 1536 global block rows
N = R * NW * K                 # 12_582_912 params
NCORES = 8
RS = R // NCORES               # 192 block rows per core
PERCORE = N // NCORES          # 1_572_864 params per core
OUT_ROWS = RS * B              # 1536 output rows per core

BR_T = 32                      # block rows per SBUF tile
NT = RS // BR_T                # 6 tiles per core
JW = 4                         # bw-groups per block row in the partition dim
BWJ = NW // JW                 # 128 block-cols per group
FREE_IN = BWJ * K              # 2048 input elems per partition per tile
U_MAX = 6                      # output rows per block that can be nonzero
FREE_OUT = U_MAX * BWJ * B     # 6144 dense output elems per partition per tile

LOG2PI = math.log(2.0 * math.pi)


def _zigzag_indices(b):
    idx = []
    for s in range(2 * b - 1):
        if s % 2 == 0:
            for i in range(s + 1):
                j = s - i
                if i < b and j < b:
                    idx.append((i, j))
        else:
            for j in range(s + 1):
                i = s - j
                if i < b and j < b:
                    idx.append((i, j))
    return idx


ZZ = _zigzag_indices(B)[:K]    # k -> (u, v)


def _runs():
    """Group the 16 (k -> (u, v)) assignments into affine runs.

    Each run is (u, v0, k0, dk, length): output positions (u, v0..v0+len) of
    every 8x8 block take coeffs k0, k0+dk, ... . v's per u are contiguous from
    0, so a run is a 2D strided copy (bw-major, k-strided)."""
    by_u = {}
    for k, (u, v) in enumerate(ZZ):
        by_u.setdefault(u, []).append((v, k))
    runs = []
    for u in sorted(by_u):
        vk = sorted(by_u[u])
        assert [v for v, _ in vk] == list(range(len(vk)))
        i = 0
        while i < len(vk):
            if i + 1 < len(vk):
                dk = vk[i + 1][1] - vk[i][1]
                j = i + 1
                while j + 1 < len(vk) and vk[j + 1][1] - vk[j][1] == dk:
                    j += 1
            else:
                dk, j = 1, i
            runs.append((u, vk[i][0], vk[i][1], dk, j - i + 1))
            i = j + 1
    assert sum(r[4] for r in runs) == K
    return runs


RUNS = _runs()


def _expected_flat_idx():
    zz = np.asarray(ZZ, dtype=np.int64)
    c = np.arange(C)[:, None, None, None]
    bh = np.arange(NH)[None, :, None, None]
    bw = np.arange(NW)[None, None, :, None]
    u = zz[:, 0][None, None, None, :]
    v = zz[:, 1][None, None, None, :]
    flat = c * (H * W) + (bh * B + u) * W + (bw * B + v)
    return flat.reshape(-1).astype(np.int32)


# ---------------------------------------------------------------- bass build
_NC = None


def _build():
    global _NC
    if _NC is not None:
        return _NC
    import concourse.bacc as bacc
    import concourse.mybir as mybir
    from concourse import tile

    f32 = mybir.dt.float32
    AF = mybir.ActivationFunctionType
    ALU = mybir.AluOpType

    # Bacc (not plain Bass): its finalize() splits multi-semaphore waits into
    # event-semaphore chains, which this walrus requires (1 wait/instruction).
    nc = bacc.Bacc("TRN2", target_bir_lowering=False)
    mean_d = nc.dram_tensor("mean", [PERCORE], f32, kind="ExternalInput")
    lstd_d = nc.dram_tensor("log_std", [PERCORE], f32, kind="ExternalInput")
    eps_d = nc.dram_tensor("eps", [PERCORE], f32, kind="ExternalInput")
    out_d = nc.dram_tensor("out", [OUT_ROWS, W], f32, kind="ExternalOutput")
    pe_d = nc.dram_tensor("p_eps2", [128, NT], f32, kind="ExternalOutput")
    pl_d = nc.dram_tensor("p_lstd", [128, NT], f32, kind="ExternalOutput")

    TILE_ELEMS = 128 * FREE_IN  # 262144 params per tile

    with tile.TileContext(nc) as tc:
        with (
            tc.tile_pool(name="ins", bufs=3) as ins,
            tc.tile_pool(name="work", bufs=2) as work,
            tc.tile_pool(name="dense", bufs=1) as dense,
            tc.tile_pool(name="acc", bufs=1) as acc,
        ):
            # Double-buffered dense staging tile; free dim = (u, bw, v).
            # Zeros are written once; scatter-adds only ever touch the
            # nonzero positions, so zeros persist across iterations.
            d_tiles = [
                dense.tile([128, FREE_OUT], f32, tag=f"D{i}", name=f"D{i}")
                for i in range(2)
            ]
            for dt_ in d_tiles:
                nc.vector.memset(dt_[:], 0.0)
            p_eps2 = acc.tile([128, NT], f32, tag="pe")
            p_lstd = acc.tile([128, NT], f32, tag="pl")
            warm = acc.tile([128, 1], f32, tag="warm")

            for t in range(NT):
                m_t = ins.tile([128, FREE_IN], f32, tag="m")
                l_t = ins.tile([128, FREE_IN], f32, tag="l")
                e_t = ins.tile([128, FREE_IN], f32, tag="e")
                # partition p = br*JW + j (contiguous load)
                src = slice(t * TILE_ELEMS, (t + 1) * TILE_ELEMS)
                nc.sync.dma_start(
                    m_t[:], mean_d[src].rearrange("(p f) -> p f", f=FREE_IN))
                nc.sync.dma_start(
                    l_t[:], lstd_d[src].rearrange("(p f) -> p f", f=FREE_IN))
                nc.sync.dma_start(
                    e_t[:], eps_d[src].rearrange("(p f) -> p f", f=FREE_IN))

                # std = exp(log_std)          (ACT)
                std_t = work.tile([128, FREE_IN], f32, tag="std")
                nc.scalar.activation(std_t[:], l_t[:], AF.Exp)
                # sum(eps^2) partial          (ACT, dead main output)
                sq_t = work.tile([128, FREE_IN], f32, tag="sq")
                nc.scalar.activation(
                    sq_t[:], e_t[:], AF.Square, accum_out=p_eps2[:, t:t + 1])
                # sum(log_std) partial        (DVE)
                nc.vector.tensor_reduce(
                    p_lstd[:, t:t + 1], l_t[:], mybir.AxisListType.X, ALU.add)
                # tmp = std * eps             (DVE)
                tmp_t = work.tile([128, FREE_IN], f32, tag="tmp")
                nc.vector.tensor_mul(tmp_t[:], std_t[:], e_t[:])

                # Make DVE observe the mean-load semaphore on a throwaway
                # copy so no scatter-add needs more than 2 sync waits
                # (TensorTensor wait capacity is 2).
                nc.vector.tensor_copy(warm[:], m_t[:, 0:1])

                # scatter: dense[u, bw, v] = tmp[bw, k] + mean[bw, k]
                d_t = d_tiles[t % 2]
                d4 = d_t[:].rearrange("p (u bw v) -> p u bw v", u=U_MAX, v=B)
                t3 = tmp_t[:].rearrange("p (bw k) -> p bw k", k=K)
                m3 = m_t[:].rearrange("p (bw k) -> p bw k", k=K)
                for (u, v0, k0, dk, ln) in RUNS:
                    ks = slice(k0, k0 + (ln - 1) * dk + 1, dk)
                    nc.vector.tensor_add(
                        d4[:, u, :, v0:v0 + ln], t3[:, :, ks], m3[:, :, ks])

                # dense rows -> HBM.  partition p = br*JW + j owns output row
                # br*8+u, cols [j*BWJ*8, (j+1)*BWJ*8).  One DMA per u: the
                # SBUF side is (br,j) partitions x 1024 (partition steps are
                # multiples of the 6144 pitch, so the AP optimizer cannot
                # fold partition dims into the free run), and on the HBM
                # side (j,c) legitimately merge into full 16KB rows.
                ov = out_d[t * BR_T * B:(t + 1) * BR_T * B, :].rearrange(
                    "(br u) (j c) -> br u j c", u=B, c=BWJ * B)
                for u in range(U_MAX):
                    nc.scalar.dma_start(ov[:, u, :, :], d4[:, u, :, :])

            nc.sync.dma_start(pe_d[:], p_eps2[:])
            nc.sync.dma_start(pl_d[:], p_lstd[:])

    nc.finalize()
    _NC = nc
    return nc


# ---------------------------------------------------------------- entry
def _scalars_from_partials(results):
    se = sum(np.asarray(r["p_eps2"], dtype=np.float64).sum() for r in results)
    sl = sum(np.asarray(r["p_lstd"], dtype=np.float64).sum() for r in results)
    log_prob = -0.5 * (se + 2.0 * sl + N * LOG2PI)
    entropy = N * 0.5 * (1.0 + LOG2PI) + sl
    return np.float32(log_prob), np.float32(entropy)


def _fallback(mean, log_std, eps, flat_idx):
    std = np.exp(log_std)
    sample = mean + std * eps
    coeffs = np.zeros(C * H * W, dtype=np.float32)
    coeffs[flat_idx] = sample
    log_prob = -0.5 * (
        np.float64((eps.astype(np.float64) ** 2).sum())
        + 2.0 * np.float64(log_std.astype(np.float64).sum()) + N * LOG2PI)
    entropy = N * 0.5 * (1.0 + LOG2PI) + np.float64(
        log_std.astype(np.float64).sum())
    return (coeffs.reshape(C, H, W), np.float32(log_prob), np.float32(entropy))


def kernel(mean, log_std, eps, flat_idx, _results_hook=None, _trace=False):
    mean = np.ascontiguousarray(np.asarray(mean, dtype=np.float32))
    log_std = np.ascontiguousarray(np.asarray(log_std, dtype=np.float32))
    eps = np.ascontiguousarray(np.asarray(eps, dtype=np.float32))
    flat_idx = np.asarray(flat_idx)

    if not np.array_equal(flat_idx, _expected_flat_idx()):
        return _fallback(mean, log_std, eps, flat_idx)

    from concourse.bass_utils import run_bass_kernel_spmd

    nc = _build()
    in_maps = [
        {
            "mean": mean[i * PERCORE:(i + 1) * PERCORE],
            "log_std": log_std[i * PERCORE:(i + 1) * PERCORE],
            "eps": eps[i * PERCORE:(i + 1) * PERCORE],
        }
        for i in range(NCORES)
    ]
    res = run_bass_kernel_spmd(
        nc, in_maps, core_ids=list(range(NCORES)), trace=_trace)
    if _results_hook is not None:
        _results_hook(res)
    results = res.results

    coeffs = np.concatenate(
        [np.asarray(r["out"]) for r in results], axis=0).reshape(C, H, W)
    log_prob, entropy = _scalars_from_partials(results)
    return coeffs, log_prob, entropy


# revision 24
# speedup vs baseline: 1.0438x; 1.0438x over previous
"""DCT-policy sample+scatter kernel for 8 TRN2 NeuronCores.

Reference computation:
    std    = exp(log_std)
    sample = mean + std * eps                       # (N,) with N = C*NH*NW*K
    coeffs = zeros(C*H*W).at[flat_idx].set(sample)  # structured zigzag scatter
    log_prob = -0.5 * sum(eps^2 + 2*log_std + log2pi)
    entropy  = sum(0.5*(1+log2pi) + log_std)

flat_idx is the deterministic zigzag-DCT layout: param (c, bh, bw, k) lands at
output [c, bh*8+u_k, bw*8+v_k] where (u_k, v_k) is the k-th zigzag position of
an 8x8 block (first 16 kept).  Treating the output as (C*NH*8, W) rows, each
block row br=(c,bh) of params (512*16 values) maps into 8 output rows; only
u=0..5 contain nonzeros, and within each 8-wide block the nonzero v's form a
contiguous run starting at v=0.  So the scatter is a fixed affine permutation:
we build the dense output tile in SBUF with a handful of strided vector adds
and DMA contiguous rows out.  Rows u=6,7 are all-zero and never written (the
runner donates zero-initialized output buffers).

Sharding: the flat param dim (and correspondingly the output rows) are split
contiguously across the 8 cores: 192 block rows of params -> 1536 output rows
per core.  Scalar sums are returned as per-partition partials and reduced on
host in float64.
"""

import math
import sys

import numpy as np

for _p in ("/opt/trn_rl_repo",):
    if _p not in sys.path:
        sys.path.insert(0, _p)

# ---------------------------------------------------------------- constants
C, H, W = 3, 4096, 4096
B = 8              # DCT block size
K = 16             # kept zigzag coeffs per block
NH, NW = H // B, W // B        # 512, 512
R = C * NH                     # 1536 global block rows
N = R * NW * K                 # 12_582_912 params
NCORES = 8
RS = R // NCORES               # 192 block rows per core
PERCORE = N // NCORES          # 1_572_864 params per core
OUT_ROWS = RS * B              # 1536 output rows per core

BR_T = 32                      # block rows per SBUF tile
NT = RS // BR_T                # 6 tiles per core
JW = 4                         # bw-groups per block row in the partition dim
BWJ = NW // JW                 # 128 block-cols per group
FREE_IN = BWJ * K              # 2048 input elems per partition per tile
U_MAX = 6                      # output rows per block that can be nonzero
FREE_OUT = U_MAX * BWJ * B     # 6144 dense output elems per partition per tile

LOG2PI = math.log(2.0 * math.pi)


def _zigzag_indices(b):
    idx = []
    for s in range(2 * b - 1):
        if s % 2 == 0:
            for i in range(s + 1):
                j = s - i
                if i < b and j < b:
                    idx.append((i, j))
        else:
            for j in range(s + 1):
                i = s - j
                if i < b and j < b:
                    idx.append((i, j))
    return idx


ZZ = _zigzag_indices(B)[:K]    # k -> (u, v)


def _runs():
    """Group the 16 (k -> (u, v)) assignments into affine runs.

    Each run is (u, v0, k0, dk, length): output positions (u, v0..v0+len) of
    every 8x8 block take coeffs k0, k0+dk, ... . v's per u are contiguous from
    0, so a run is a 2D strided copy (bw-major, k-strided)."""
    by_u = {}
    for k, (u, v) in enumerate(ZZ):
        by_u.setdefault(u, []).append((v, k))
    runs = []
    for u in sorted(by_u):
        vk = sorted(by_u[u])
        assert [v for v, _ in vk] == list(range(len(vk)))
        i = 0
        while i < len(vk):
            if i + 1 < len(vk):
                dk = vk[i + 1][1] - vk[i][1]
                j = i + 1
                while j + 1 < len(vk) and vk[j + 1][1] - vk[j][1] == dk:
                    j += 1
            else:
                dk, j = 1, i
            runs.append((u, vk[i][0], vk[i][1], dk, j - i + 1))
            i = j + 1
    assert sum(r[4] for r in runs) == K
    return runs


RUNS = _runs()


def _expected_flat_idx():
    zz = np.asarray(ZZ, dtype=np.int64)
    c = np.arange(C)[:, None, None, None]
    bh = np.arange(NH)[None, :, None, None]
    bw = np.arange(NW)[None, None, :, None]
    u = zz[:, 0][None, None, None, :]
    v = zz[:, 1][None, None, None, :]
    flat = c * (H * W) + (bh * B + u) * W + (bw * B + v)
    return flat.reshape(-1).astype(np.int32)


# ---------------------------------------------------------------- bass build
_NC = None


def _build():
    global _NC
    if _NC is not None:
        return _NC
    import concourse.bacc as bacc
    import concourse.mybir as mybir
    from concourse import tile

    f32 = mybir.dt.float32
    AF = mybir.ActivationFunctionType
    ALU = mybir.AluOpType

    # Bacc (not plain Bass): its finalize() splits multi-semaphore waits into
    # event-semaphore chains, which this walrus requires (1 wait/instruction).
    nc = bacc.Bacc("TRN2", target_bir_lowering=False)
    mean_d = nc.dram_tensor("mean", [PERCORE], f32, kind="ExternalInput")
    lstd_d = nc.dram_tensor("log_std", [PERCORE], f32, kind="ExternalInput")
    eps_d = nc.dram_tensor("eps", [PERCORE], f32, kind="ExternalInput")
    out_d = nc.dram_tensor("out", [OUT_ROWS, W], f32, kind="ExternalOutput")
    pe_d = nc.dram_tensor("p_eps2", [128, NT], f32, kind="ExternalOutput")
    pl_d = nc.dram_tensor("p_lstd", [128, NT], f32, kind="ExternalOutput")

    TILE_ELEMS = 128 * FREE_IN  # 262144 params per tile

    with tile.TileContext(nc) as tc:
        with (
            tc.tile_pool(name="ins", bufs=3) as ins,
            tc.tile_pool(name="work", bufs=2) as work,
            tc.tile_pool(name="dense", bufs=1) as dense,
            tc.tile_pool(name="acc", bufs=1) as acc,
        ):
            # Double-buffered dense staging tile; free dim = (u, bw, v).
            # Zeros are written once; scatter-adds only ever touch the
            # nonzero positions, so zeros persist across iterations.
            d_tiles = [
                dense.tile([128, FREE_OUT], f32, tag=f"D{i}", name=f"D{i}")
                for i in range(2)
            ]
            for dt_ in d_tiles:
                nc.gpsimd.memset(dt_[:], 0.0)
            p_eps2 = acc.tile([128, NT], f32, tag="pe")
            p_lstd = acc.tile([128, NT], f32, tag="pl")
            warm = acc.tile([128, 1], f32, tag="warm")

            for t in range(NT):
                m_t = ins.tile([128, FREE_IN], f32, tag="m")
                l_t = ins.tile([128, FREE_IN], f32, tag="l")
                e_t = ins.tile([128, FREE_IN], f32, tag="e")
                # partition p = br*JW + j (contiguous load)
                src = slice(t * TILE_ELEMS, (t + 1) * TILE_ELEMS)
                nc.sync.dma_start(
                    m_t[:], mean_d[src].rearrange("(p f) -> p f", f=FREE_IN))
                nc.sync.dma_start(
                    l_t[:], lstd_d[src].rearrange("(p f) -> p f", f=FREE_IN))
                nc.sync.dma_start(
                    e_t[:], eps_d[src].rearrange("(p f) -> p f", f=FREE_IN))

                # std = exp(log_std)          (ACT)
                std_t = work.tile([128, FREE_IN], f32, tag="std")
                nc.scalar.activation(std_t[:], l_t[:], AF.Exp)
                # sum(eps^2) partial          (ACT, dead main output)
                sq_t = work.tile([128, FREE_IN], f32, tag="sq")
                nc.scalar.activation(
                    sq_t[:], e_t[:], AF.Square, accum_out=p_eps2[:, t:t + 1])
                # sum(log_std) partial        (DVE)
                nc.vector.tensor_reduce(
                    p_lstd[:, t:t + 1], l_t[:], mybir.AxisListType.X, ALU.add)
                # tmp = std * eps             (DVE)
                tmp_t = work.tile([128, FREE_IN], f32, tag="tmp")
                nc.vector.tensor_mul(tmp_t[:], std_t[:], e_t[:])

                # Make DVE observe the mean-load semaphore on a throwaway
                # copy so no scatter-add needs more than 2 sync waits
                # (TensorTensor wait capacity is 2).
                nc.vector.tensor_copy(warm[:], m_t[:, 0:1])

                # scatter: dense[u, bw, v] = tmp[bw, k] + mean[bw, k]
                d_t = d_tiles[t % 2]
                d4 = d_t[:].rearrange("p (u bw v) -> p u bw v", u=U_MAX, v=B)
                t3 = tmp_t[:].rearrange("p (bw k) -> p bw k", k=K)
                m3 = m_t[:].rearrange("p (bw k) -> p bw k", k=K)
                for (u, v0, k0, dk, ln) in RUNS:
                    ks = slice(k0, k0 + (ln - 1) * dk + 1, dk)
                    nc.vector.tensor_add(
                        d4[:, u, :, v0:v0 + ln], t3[:, :, ks], m3[:, :, ks])

                # dense rows -> HBM.  partition p = br*JW + j owns output row
                # br*8+u, cols [j*BWJ*8, (j+1)*BWJ*8).  One DMA per u: the
                # SBUF side is (br,j) partitions x 1024 (partition steps are
                # multiples of the 6144 pitch, so the AP optimizer cannot
                # fold partition dims into the free run), and on the HBM
                # side (j,c) legitimately merge into full 16KB rows.
                ov = out_d[t * BR_T * B:(t + 1) * BR_T * B, :].rearrange(
                    "(br u) (j c) -> br u j c", u=B, c=BWJ * B)
                for u in range(U_MAX):
                    nc.scalar.dma_start(ov[:, u, :, :], d4[:, u, :, :])

            nc.sync.dma_start(pe_d[:], p_eps2[:])
            nc.sync.dma_start(pl_d[:], p_lstd[:])

    nc.finalize()
    _NC = nc
    return nc


# ---------------------------------------------------------------- entry
def _scalars_from_partials(results):
    se = sum(np.asarray(r["p_eps2"], dtype=np.float64).sum() for r in results)
    sl = sum(np.asarray(r["p_lstd"], dtype=np.float64).sum() for r in results)
    log_prob = -0.5 * (se + 2.0 * sl + N * LOG2PI)
    entropy = N * 0.5 * (1.0 + LOG2PI) + sl
    return np.float32(log_prob), np.float32(entropy)


def _fallback(mean, log_std, eps, flat_idx):
    std = np.exp(log_std)
    sample = mean + std * eps
    coeffs = np.zeros(C * H * W, dtype=np.float32)
    coeffs[flat_idx] = sample
    log_prob = -0.5 * (
        np.float64((eps.astype(np.float64) ** 2).sum())
        + 2.0 * np.float64(log_std.astype(np.float64).sum()) + N * LOG2PI)
    entropy = N * 0.5 * (1.0 + LOG2PI) + np.float64(
        log_std.astype(np.float64).sum())
    return (coeffs.reshape(C, H, W), np.float32(log_prob), np.float32(entropy))


def kernel(mean, log_std, eps, flat_idx, _results_hook=None, _trace=False):
    mean = np.ascontiguousarray(np.asarray(mean, dtype=np.float32))
    log_std = np.ascontiguousarray(np.asarray(log_std, dtype=np.float32))
    eps = np.ascontiguousarray(np.asarray(eps, dtype=np.float32))
    flat_idx = np.asarray(flat_idx)

    if not np.array_equal(flat_idx, _expected_flat_idx()):
        return _fallback(mean, log_std, eps, flat_idx)

    from concourse.bass_utils import run_bass_kernel_spmd

    nc = _build()
    in_maps = [
        {
            "mean": mean[i * PERCORE:(i + 1) * PERCORE],
            "log_std": log_std[i * PERCORE:(i + 1) * PERCORE],
            "eps": eps[i * PERCORE:(i + 1) * PERCORE],
        }
        for i in range(NCORES)
    ]
    res = run_bass_kernel_spmd(
        nc, in_maps, core_ids=list(range(NCORES)), trace=_trace)
    if _results_hook is not None:
        _results_hook(res)
    results = res.results

    coeffs = np.concatenate(
        [np.asarray(r["out"]) for r in results], axis=0).reshape(C, H, W)
    log_prob, entropy = _scalars_from_partials(results)
    return coeffs, log_prob, entropy


# revision 26
# speedup vs baseline: 1.1755x; 1.1262x over previous
"""DCT-policy sample+scatter kernel for 8 TRN2 NeuronCores.

Reference computation:
    std    = exp(log_std)
    sample = mean + std * eps                       # (N,) with N = C*NH*NW*K
    coeffs = zeros(C*H*W).at[flat_idx].set(sample)  # structured zigzag scatter
    log_prob = -0.5 * sum(eps^2 + 2*log_std + log2pi)
    entropy  = sum(0.5*(1+log2pi) + log_std)

flat_idx is the deterministic zigzag-DCT layout: param (c, bh, bw, k) lands at
output [c, bh*8+u_k, bw*8+v_k] where (u_k, v_k) is the k-th zigzag position of
an 8x8 block (first 16 kept).  Treating the output as (C*NH*8, W) rows, each
block row br=(c,bh) of params (512*16 values) maps into 8 output rows; only
u=0..5 contain nonzeros, and within each 8-wide block the nonzero v's form a
contiguous run starting at v=0.  So the scatter is a fixed affine permutation:
we build the dense output tile in SBUF with a handful of strided vector adds
and DMA contiguous rows out.  Rows u=6,7 are all-zero and never written (the
runner donates zero-initialized output buffers).

Sharding: the flat param dim (and correspondingly the output rows) are split
contiguously across the 8 cores: 192 block rows of params -> 1536 output rows
per core.  Scalar sums are returned as per-partition partials and reduced on
host in float64.
"""

import math
import sys

import numpy as np

for _p in ("/opt/trn_rl_repo",):
    if _p not in sys.path:
        sys.path.insert(0, _p)

# ---------------------------------------------------------------- constants
C, H, W = 3, 4096, 4096
B = 8              # DCT block size
K = 16             # kept zigzag coeffs per block
NH, NW = H // B, W // B        # 512, 512
R = C * NH                     # 1536 global block rows
N = R * NW * K                 # 12_582_912 params
NCORES = 8
RS = R // NCORES               # 192 block rows per core
PERCORE = N // NCORES          # 1_572_864 params per core
OUT_ROWS = RS * B              # 1536 output rows per core

BR_T = 32                      # block rows per SBUF tile
NT = RS // BR_T                # 6 tiles per core
JW = 4                         # bw-groups per block row in the partition dim
BWJ = NW // JW                 # 128 block-cols per group
FREE_IN = BWJ * K              # 2048 input elems per partition per tile
U_MAX = 6                      # output rows per block that can be nonzero
FREE_OUT = U_MAX * BWJ * B     # 6144 dense output elems per partition per tile

LOG2PI = math.log(2.0 * math.pi)


def _zigzag_indices(b):
    idx = []
    for s in range(2 * b - 1):
        if s % 2 == 0:
            for i in range(s + 1):
                j = s - i
                if i < b and j < b:
                    idx.append((i, j))
        else:
            for j in range(s + 1):
                i = s - j
                if i < b and j < b:
                    idx.append((i, j))
    return idx


ZZ = _zigzag_indices(B)[:K]    # k -> (u, v)


def _runs():
    """Group the 16 (k -> (u, v)) assignments into affine runs.

    Each run is (u, v0, k0, dk, length): output positions (u, v0..v0+len) of
    every 8x8 block take coeffs k0, k0+dk, ... . v's per u are contiguous from
    0, so a run is a 2D strided copy (bw-major, k-strided)."""
    by_u = {}
    for k, (u, v) in enumerate(ZZ):
        by_u.setdefault(u, []).append((v, k))
    runs = []
    for u in sorted(by_u):
        vk = sorted(by_u[u])
        assert [v for v, _ in vk] == list(range(len(vk)))
        i = 0
        while i < len(vk):
            if i + 1 < len(vk):
                dk = vk[i + 1][1] - vk[i][1]
                j = i + 1
                while j + 1 < len(vk) and vk[j + 1][1] - vk[j][1] == dk:
                    j += 1
            else:
                dk, j = 1, i
            runs.append((u, vk[i][0], vk[i][1], dk, j - i + 1))
            i = j + 1
    assert sum(r[4] for r in runs) == K
    return runs


RUNS = _runs()


def _expected_flat_idx():
    zz = np.asarray(ZZ, dtype=np.int64)
    c = np.arange(C)[:, None, None, None]
    bh = np.arange(NH)[None, :, None, None]
    bw = np.arange(NW)[None, None, :, None]
    u = zz[:, 0][None, None, None, :]
    v = zz[:, 1][None, None, None, :]
    flat = c * (H * W) + (bh * B + u) * W + (bw * B + v)
    return flat.reshape(-1).astype(np.int32)


# ---------------------------------------------------------------- bass build
_NC = None


def _build():
    global _NC
    if _NC is not None:
        return _NC
    import concourse.bacc as bacc
    import concourse.mybir as mybir
    from concourse import tile

    f32 = mybir.dt.float32
    AF = mybir.ActivationFunctionType
    ALU = mybir.AluOpType

    # Bacc (not plain Bass): its finalize() splits multi-semaphore waits into
    # event-semaphore chains, which this walrus requires (1 wait/instruction).
    nc = bacc.Bacc("TRN2", target_bir_lowering=False)
    mean_d = nc.dram_tensor("mean", [PERCORE], f32, kind="ExternalInput")
    lstd_d = nc.dram_tensor("log_std", [PERCORE], f32, kind="ExternalInput")
    eps_d = nc.dram_tensor("eps", [PERCORE], f32, kind="ExternalInput")
    out_d = nc.dram_tensor("out", [OUT_ROWS, W], f32, kind="ExternalOutput")
    pe_d = nc.dram_tensor("p_eps2", [128, NT], f32, kind="ExternalOutput")
    pl_d = nc.dram_tensor("p_lstd", [128, NT], f32, kind="ExternalOutput")

    TILE_ELEMS = 128 * FREE_IN  # 262144 params per tile

    with tile.TileContext(nc) as tc:
        with (
            tc.tile_pool(name="ins", bufs=3) as ins,
            tc.tile_pool(name="work", bufs=2) as work,
            tc.tile_pool(name="dense", bufs=1) as dense,
            tc.tile_pool(name="acc", bufs=1) as acc,
        ):
            # Double-buffered dense staging tile; free dim = (u, bw, v).
            # Zeros are written once; scatter-adds only ever touch the
            # nonzero positions, so zeros persist across iterations.
            d_tiles = [
                dense.tile([128, FREE_OUT], f32, tag=f"D{i}", name=f"D{i}")
                for i in range(2)
            ]
            for dt_ in d_tiles:
                nc.gpsimd.memset(dt_[:], 0.0)
            p_eps2 = acc.tile([128, NT], f32, tag="pe")
            p_lstd = acc.tile([128, NT], f32, tag="pl")
            warm = acc.tile([128, 1], f32, tag="warm")

            big = {}
            for t in range(NT):
                if t % 2 == 0:
                    # 1MB double-tile loads (amortize per-DMA overhead).
                    # Partition p gets [tile t chunk p | tile t+1 chunk p].
                    for nm_, dram in (("m", mean_d), ("l", lstd_d),
                                      ("e", eps_d)):
                        b_t = ins.tile([128, 2 * FREE_IN], f32, tag=nm_,
                                       name=f"{nm_}{t}", bufs=2)
                        src = dram[t * TILE_ELEMS:(t + 2) * TILE_ELEMS]
                        nc.sync.dma_start(
                            b_t[:], src.rearrange("(tt p f) -> p tt f",
                                                  tt=2, f=FREE_IN))
                        big[nm_] = b_t
                half = slice((t % 2) * FREE_IN, (t % 2 + 1) * FREE_IN)
                m_t = big["m"][:, half]
                l_t = big["l"][:, half]
                e_t = big["e"][:, half]

                # std = exp(log_std)          (ACT)
                std_t = work.tile([128, FREE_IN], f32, tag="std")
                nc.scalar.activation(std_t[:], l_t, AF.Exp)
                # sum(eps^2) partial          (ACT, dead main output)
                sq_t = work.tile([128, FREE_IN], f32, tag="sq", bufs=1)
                nc.scalar.activation(
                    sq_t[:], e_t, AF.Square, accum_out=p_eps2[:, t:t + 1])
                # sum(log_std) partial        (ACT, dead main output)
                nc.scalar.activation(
                    sq_t[:], l_t, AF.Identity,
                    accum_out=p_lstd[:, t:t + 1])
                # tmp = std * eps             (DVE)
                tmp_t = work.tile([128, FREE_IN], f32, tag="tmp")
                nc.vector.tensor_mul(tmp_t[:], std_t[:], e_t)

                # Make DVE observe the mean-load semaphore on a throwaway
                # copy so no scatter-add needs more than 2 sync waits
                # (TensorTensor wait capacity is 2).
                nc.vector.tensor_copy(warm[:], m_t[:, 0:1])

                # scatter: dense[u, bw, v] = tmp[bw, k] + mean[bw, k]
                # u>=3 runs go to the otherwise-idle GpSimd engine.
                d_t = d_tiles[t % 2]
                d4 = d_t[:].rearrange("p (u bw v) -> p u bw v", u=U_MAX, v=B)
                t3 = tmp_t[:].rearrange("p (bw k) -> p bw k", k=K)
                m3 = m_t.rearrange("p (bw k) -> p bw k", k=K)
                for (u, v0, k0, dk, ln) in RUNS:
                    ks = slice(k0, k0 + (ln - 1) * dk + 1, dk)
                    eng = nc.gpsimd if u >= 3 else nc.vector
                    eng.tensor_add(
                        d4[:, u, :, v0:v0 + ln], t3[:, :, ks], m3[:, :, ks])

                # dense rows -> HBM.  partition p = br*JW + j owns output row
                # br*8+u, cols [j*BWJ*8, (j+1)*BWJ*8).  One DMA per u: the
                # SBUF side is (br,j) partitions x 1024 (partition steps are
                # multiples of the 6144 pitch, so the AP optimizer cannot
                # fold partition dims into the free run), and on the HBM
                # side (j,c) legitimately merge into full 16KB rows.
                ov = out_d[t * BR_T * B:(t + 1) * BR_T * B, :].rearrange(
                    "(br u) (j c) -> br u j c", u=B, c=BWJ * B)
                for u in range(U_MAX):
                    nc.scalar.dma_start(ov[:, u, :, :], d4[:, u, :, :])

            nc.sync.dma_start(pe_d[:], p_eps2[:])
            nc.sync.dma_start(pl_d[:], p_lstd[:])

    nc.finalize()
    _NC = nc
    return nc


# ---------------------------------------------------------------- entry
def _scalars_from_partials(results):
    se = sum(np.asarray(r["p_eps2"], dtype=np.float64).sum() for r in results)
    sl = sum(np.asarray(r["p_lstd"], dtype=np.float64).sum() for r in results)
    log_prob = -0.5 * (se + 2.0 * sl + N * LOG2PI)
    entropy = N * 0.5 * (1.0 + LOG2PI) + sl
    return np.float32(log_prob), np.float32(entropy)


def _fallback(mean, log_std, eps, flat_idx):
    std = np.exp(log_std)
    sample = mean + std * eps
    coeffs = np.zeros(C * H * W, dtype=np.float32)
    coeffs[flat_idx] = sample
    log_prob = -0.5 * (
        np.float64((eps.astype(np.float64) ** 2).sum())
        + 2.0 * np.float64(log_std.astype(np.float64).sum()) + N * LOG2PI)
    entropy = N * 0.5 * (1.0 + LOG2PI) + np.float64(
        log_std.astype(np.float64).sum())
    return (coeffs.reshape(C, H, W), np.float32(log_prob), np.float32(entropy))


def kernel(mean, log_std, eps, flat_idx, _results_hook=None, _trace=False):
    mean = np.ascontiguousarray(np.asarray(mean, dtype=np.float32))
    log_std = np.ascontiguousarray(np.asarray(log_std, dtype=np.float32))
    eps = np.ascontiguousarray(np.asarray(eps, dtype=np.float32))
    flat_idx = np.asarray(flat_idx)

    if not np.array_equal(flat_idx, _expected_flat_idx()):
        return _fallback(mean, log_std, eps, flat_idx)

    from concourse.bass_utils import run_bass_kernel_spmd

    nc = _build()
    in_maps = [
        {
            "mean": mean[i * PERCORE:(i + 1) * PERCORE],
            "log_std": log_std[i * PERCORE:(i + 1) * PERCORE],
            "eps": eps[i * PERCORE:(i + 1) * PERCORE],
        }
        for i in range(NCORES)
    ]
    res = run_bass_kernel_spmd(
        nc, in_maps, core_ids=list(range(NCORES)), trace=_trace)
    if _results_hook is not None:
        _results_hook(res)
    results = res.results

    coeffs = np.concatenate(
        [np.asarray(r["out"]) for r in results], axis=0).reshape(C, H, W)
    log_prob, entropy = _scalars_from_partials(results)
    return coeffs, log_prob, entropy


# revision 30
# speedup vs baseline: 1.1801x; 1.0039x over previous
"""DCT-policy sample+scatter kernel for 8 TRN2 NeuronCores.

Reference computation:
    std    = exp(log_std)
    sample = mean + std * eps                       # (N,) with N = C*NH*NW*K
    coeffs = zeros(C*H*W).at[flat_idx].set(sample)  # structured zigzag scatter
    log_prob = -0.5 * sum(eps^2 + 2*log_std + log2pi)
    entropy  = sum(0.5*(1+log2pi) + log_std)

flat_idx is the deterministic zigzag-DCT layout: param (c, bh, bw, k) lands at
output [c, bh*8+u_k, bw*8+v_k] where (u_k, v_k) is the k-th zigzag position of
an 8x8 block (first 16 kept).  Treating the output as (C*NH*8, W) rows, each
block row br=(c,bh) of params (512*16 values) maps into 8 output rows; only
u=0..5 contain nonzeros, and within each 8-wide block the nonzero v's form a
contiguous run starting at v=0.  So the scatter is a fixed affine permutation:
we build the dense output tile in SBUF with a handful of strided vector adds
and DMA contiguous rows out.  Rows u=6,7 are all-zero and never written (the
runner donates zero-initialized output buffers).

Sharding: the flat param dim (and correspondingly the output rows) are split
contiguously across the 8 cores: 192 block rows of params -> 1536 output rows
per core.  Scalar sums are returned as per-partition partials and reduced on
host in float64.
"""

import math
import sys

import numpy as np

for _p in ("/opt/trn_rl_repo",):
    if _p not in sys.path:
        sys.path.insert(0, _p)

# ---------------------------------------------------------------- constants
C, H, W = 3, 4096, 4096
B = 8              # DCT block size
K = 16             # kept zigzag coeffs per block
NH, NW = H // B, W // B        # 512, 512
R = C * NH                     # 1536 global block rows
N = R * NW * K                 # 12_582_912 params
NCORES = 8
RS = R // NCORES               # 192 block rows per core
PERCORE = N // NCORES          # 1_572_864 params per core
OUT_ROWS = RS * B              # 1536 output rows per core

BR_T = 32                      # block rows per SBUF tile
NT = RS // BR_T                # 6 tiles per core
JW = 4                         # bw-groups per block row in the partition dim
BWJ = NW // JW                 # 128 block-cols per group
FREE_IN = BWJ * K              # 2048 input elems per partition per tile
U_MAX = 6                      # output rows per block that can be nonzero
FREE_OUT = U_MAX * BWJ * B     # 6144 dense output elems per partition per tile

LOG2PI = math.log(2.0 * math.pi)


def _zigzag_indices(b):
    idx = []
    for s in range(2 * b - 1):
        if s % 2 == 0:
            for i in range(s + 1):
                j = s - i
                if i < b and j < b:
                    idx.append((i, j))
        else:
            for j in range(s + 1):
                i = s - j
                if i < b and j < b:
                    idx.append((i, j))
    return idx


ZZ = _zigzag_indices(B)[:K]    # k -> (u, v)


def _runs():
    """Group the 16 (k -> (u, v)) assignments into affine runs.

    Each run is (u, v0, k0, dk, length): output positions (u, v0..v0+len) of
    every 8x8 block take coeffs k0, k0+dk, ... . v's per u are contiguous from
    0, so a run is a 2D strided copy (bw-major, k-strided)."""
    by_u = {}
    for k, (u, v) in enumerate(ZZ):
        by_u.setdefault(u, []).append((v, k))
    runs = []
    for u in sorted(by_u):
        vk = sorted(by_u[u])
        assert [v for v, _ in vk] == list(range(len(vk)))
        i = 0
        while i < len(vk):
            if i + 1 < len(vk):
                dk = vk[i + 1][1] - vk[i][1]
                j = i + 1
                while j + 1 < len(vk) and vk[j + 1][1] - vk[j][1] == dk:
                    j += 1
            else:
                dk, j = 1, i
            runs.append((u, vk[i][0], vk[i][1], dk, j - i + 1))
            i = j + 1
    assert sum(r[4] for r in runs) == K
    return runs


RUNS = _runs()


def _expected_flat_idx():
    zz = np.asarray(ZZ, dtype=np.int64)
    c = np.arange(C)[:, None, None, None]
    bh = np.arange(NH)[None, :, None, None]
    bw = np.arange(NW)[None, None, :, None]
    u = zz[:, 0][None, None, None, :]
    v = zz[:, 1][None, None, None, :]
    flat = c * (H * W) + (bh * B + u) * W + (bw * B + v)
    return flat.reshape(-1).astype(np.int32)


# ---------------------------------------------------------------- bass build
_NC = None


def _build():
    global _NC
    if _NC is not None:
        return _NC
    import concourse.bacc as bacc
    import concourse.mybir as mybir
    from concourse import tile

    f32 = mybir.dt.float32
    AF = mybir.ActivationFunctionType
    ALU = mybir.AluOpType

    # Bacc (not plain Bass): its finalize() splits multi-semaphore waits into
    # event-semaphore chains, which this walrus requires (1 wait/instruction).
    nc = bacc.Bacc("TRN2", target_bir_lowering=False)
    mean_d = nc.dram_tensor("mean", [PERCORE], f32, kind="ExternalInput")
    lstd_d = nc.dram_tensor("log_std", [PERCORE], f32, kind="ExternalInput")
    eps_d = nc.dram_tensor("eps", [PERCORE], f32, kind="ExternalInput")
    out_d = nc.dram_tensor("out", [OUT_ROWS, W], f32, kind="ExternalOutput")
    pe_d = nc.dram_tensor("p_eps2", [128, NT], f32, kind="ExternalOutput")
    pl_d = nc.dram_tensor("p_lstd", [128, NT], f32, kind="ExternalOutput")

    TILE_ELEMS = 128 * FREE_IN  # 262144 params per tile

    with tile.TileContext(nc) as tc:
        with (
            tc.tile_pool(name="ins", bufs=3) as ins,
            tc.tile_pool(name="work", bufs=2) as work,
            tc.tile_pool(name="dense", bufs=1) as dense,
            tc.tile_pool(name="acc", bufs=1) as acc,
        ):
            # Double-buffered dense staging tile; free dim = (u, bw, v).
            # Zeros are written once; scatter-adds only ever touch the
            # nonzero positions, so zeros persist across iterations.
            d_tiles = [
                dense.tile([128, FREE_OUT], f32, tag=f"D{i}", name=f"D{i}")
                for i in range(2)
            ]
            for dt_ in d_tiles:
                nc.gpsimd.memset(dt_[:], 0.0)
            p_eps2 = acc.tile([128, NT], f32, tag="pe")
            p_lstd = acc.tile([128, NT], f32, tag="pl")
            warm = acc.tile([128, 1], f32, tag="warm")

            big = {}
            for t in range(NT):
                if t % 2 == 0:
                    # 1MB double-tile loads (amortize per-DMA overhead).
                    # Partition p gets [tile t chunk p | tile t+1 chunk p].
                    for nm_, dram in (("m", mean_d), ("l", lstd_d),
                                      ("e", eps_d)):
                        b_t = ins.tile([128, 2 * FREE_IN], f32, tag=nm_,
                                       name=f"{nm_}{t}", bufs=2)
                        src = dram[t * TILE_ELEMS:(t + 2) * TILE_ELEMS]
                        nc.sync.dma_start(
                            b_t[:], src.rearrange("(tt p f) -> p tt f",
                                                  tt=2, f=FREE_IN))
                        big[nm_] = b_t
                half = slice((t % 2) * FREE_IN, (t % 2 + 1) * FREE_IN)
                m_t = big["m"][:, half]
                l_t = big["l"][:, half]
                e_t = big["e"][:, half]

                # std = exp(log_std)          (ACT)
                std_t = work.tile([128, FREE_IN], f32, tag="std")
                nc.scalar.activation(std_t[:], l_t, AF.Exp)
                # sum(eps^2) partial          (ACT, dead main output)
                sq_t = work.tile([128, FREE_IN], f32, tag="sq", bufs=1)
                nc.scalar.activation(
                    sq_t[:], e_t, AF.Square, accum_out=p_eps2[:, t:t + 1])
                # sum(log_std) partial        (ACT, dead main output)
                nc.scalar.activation(
                    sq_t[:], l_t, AF.Identity,
                    accum_out=p_lstd[:, t:t + 1])
                # tmp = std * eps             (DVE)
                tmp_t = work.tile([128, FREE_IN], f32, tag="tmp")
                nc.vector.tensor_mul(tmp_t[:], std_t[:], e_t)

                # Make DVE observe the mean-load semaphore on a throwaway
                # copy so no scatter-add needs more than 2 sync waits
                # (TensorTensor wait capacity is 2).
                nc.vector.tensor_copy(warm[:], m_t[:, 0:1])

                # scatter: dense[u, bw, v] = tmp[bw, k] + mean[bw, k]
                # u>=3 runs go to the otherwise-idle GpSimd engine.
                d_t = d_tiles[t % 2]
                d4 = d_t[:].rearrange("p (u bw v) -> p u bw v", u=U_MAX, v=B)
                t3 = tmp_t[:].rearrange("p (bw k) -> p bw k", k=K)
                m3 = m_t.rearrange("p (bw k) -> p bw k", k=K)
                for (u, v0, k0, dk, ln) in RUNS:
                    ks = slice(k0, k0 + (ln - 1) * dk + 1, dk)
                    eng = nc.gpsimd if u >= 3 else nc.vector
                    eng.tensor_add(
                        d4[:, u, :, v0:v0 + ln], t3[:, :, ks], m3[:, :, ks])

                # dense rows -> HBM.  partition p = br*JW + j owns output row
                # br*8+u, cols [j*BWJ*8, (j+1)*BWJ*8).  One DMA per u: the
                # SBUF side is (br,j) partitions x 1024 (partition steps are
                # multiples of the 6144 pitch, so the AP optimizer cannot
                # fold partition dims into the free run), and on the HBM
                # side (j,c) legitimately merge into full 16KB rows.
                ov = out_d[t * BR_T * B:(t + 1) * BR_T * B, :].rearrange(
                    "(br u) (j c) -> br u j c", u=B, c=BWJ * B)
                for u in range(U_MAX):
                    nc.scalar.dma_start(ov[:, u, :, :], d4[:, u, :, :])

            nc.sync.dma_start(pe_d[:], p_eps2[:])
            nc.sync.dma_start(pl_d[:], p_lstd[:])

    nc.finalize()
    _NC = nc
    return nc


# ---------------------------------------------------------------- entry
def _scalars_from_partials(results):
    se = sum(np.asarray(r["p_eps2"], dtype=np.float64).sum() for r in results)
    sl = sum(np.asarray(r["p_lstd"], dtype=np.float64).sum() for r in results)
    log_prob = -0.5 * (se + 2.0 * sl + N * LOG2PI)
    entropy = N * 0.5 * (1.0 + LOG2PI) + sl
    return np.float32(log_prob), np.float32(entropy)


def _fallback(mean, log_std, eps, flat_idx):
    std = np.exp(log_std)
    sample = mean + std * eps
    coeffs = np.zeros(C * H * W, dtype=np.float32)
    coeffs[flat_idx] = sample
    log_prob = -0.5 * (
        np.float64((eps.astype(np.float64) ** 2).sum())
        + 2.0 * np.float64(log_std.astype(np.float64).sum()) + N * LOG2PI)
    entropy = N * 0.5 * (1.0 + LOG2PI) + np.float64(
        log_std.astype(np.float64).sum())
    return (coeffs.reshape(C, H, W), np.float32(log_prob), np.float32(entropy))


def kernel(mean, log_std, eps, flat_idx, _results_hook=None, _trace=False):
    mean = np.ascontiguousarray(np.asarray(mean, dtype=np.float32))
    log_std = np.ascontiguousarray(np.asarray(log_std, dtype=np.float32))
    eps = np.ascontiguousarray(np.asarray(eps, dtype=np.float32))
    flat_idx = np.asarray(flat_idx)

    if not np.array_equal(flat_idx, _expected_flat_idx()):
        return _fallback(mean, log_std, eps, flat_idx)

    from concourse.bass_utils import run_bass_kernel_spmd

    nc = _build()
    in_maps = [
        {
            "mean": mean[i * PERCORE:(i + 1) * PERCORE],
            "log_std": log_std[i * PERCORE:(i + 1) * PERCORE],
            "eps": eps[i * PERCORE:(i + 1) * PERCORE],
        }
        for i in range(NCORES)
    ]
    res = run_bass_kernel_spmd(
        nc, in_maps, core_ids=list(range(NCORES)), trace=_trace)
    if _results_hook is not None:
        _results_hook(res)
    results = res.results

    coeffs = np.concatenate(
        [np.asarray(r["out"]) for r in results], axis=0).reshape(C, H, W)
    log_prob, entropy = _scalars_from_partials(results)
    return coeffs, log_prob, entropy
